# revision 4
# baseline (speedup 1.0000x reference)
"""Trainium2 Bass kernel for nn_Conv1DTokenEncoder.

Math: reference computes, per (b,t) row of length L=1024,
  out[b,t,d] = (1/L) * sum_k w[d,k] * S[b,t,k] + bias[d]
with S[b,t,k] = windowed sums of the zero-padded row. For K=5, pad=2:
  S[0] = total - x[L-2] - x[L-1]
  S[1] = total - x[L-1]
  S[2] = total
  S[3] = total - x[0]
  S[4] = total - x[0] - x[1]
so with host-precomputed M6 [6, D]:
  out[r, :] = [total, x0, x1, xL2, xL1, 1] @ M6
where M6 rows are [wsum/L, -(w3+w4)/L, -w4/L, -w0/L, -(w0+w1)/L, bias].

Device work per 128-row tile: row-sum on ScalarE (activation accum),
edge-column copies on VectorE, tiny PE transpose ([128,6] -> [6,128]) and a
K=6 matmul ([6,128]^T @ [6,512]) producing [128,512], then DMA out.
Pure data parallel across 8 cores (batch*token rows sharded).
"""

import numpy as np

B, T, L, D = 16, 2048, 1024, 512
N_CORES = 8
BT = B * T
ROWS_PER_CORE = BT // N_CORES  # 4096
P = 128
N_TILES = ROWS_PER_CORE // P   # 32

_CACHE = {}


def _build(repeat: int = 1):
    import concourse.bass as bass
    import concourse.tile as tile
    from concourse import bacc, mybir

    f32 = mybir.dt.float32
    nc = bacc.Bacc("TRN2", target_bir_lowering=False, debug=False)

    x_d = nc.dram_tensor("x", [ROWS_PER_CORE, L], f32, kind="ExternalInput")
    m_d = nc.dram_tensor("m6", [6, D], f32, kind="ExternalInput")
    id_d = nc.dram_tensor("ident", [P, P], f32, kind="ExternalInput")
    o_d = nc.dram_tensor("out", [ROWS_PER_CORE, D], f32, kind="ExternalOutput")

    AF = mybir.ActivationFunctionType

    with tile.TileContext(nc) as tc:
        with (
            tc.tile_pool(name="const", bufs=1) as constp,
            tc.tile_pool(name="xin", bufs=4) as xin,
            tc.tile_pool(name="scratch", bufs=2) as scratchp,
            tc.tile_pool(name="feat", bufs=3) as featp,
            tc.tile_pool(name="ftT_ps", bufs=2, space="PSUM") as ftp,
            tc.tile_pool(name="ftT_sb", bufs=2) as fts,
            tc.tile_pool(name="out_ps", bufs=2, space="PSUM") as outp,
            tc.tile_pool(name="out_sb", bufs=3) as outs,
        ):
            m6 = constp.tile([6, D], f32)
            nc.sync.dma_start(m6[:], m_d[:])
            ident = constp.tile([P, P], f32)
            nc.sync.dma_start(ident[:], id_d[:])

            def body():
                for i in range(N_TILES):
                    xt = xin.tile([P, L], f32)
                    nc.sync.dma_start(xt[:], x_d[bass.ts(i, P), :])

                    ft = featp.tile([P, 6], f32)
                    sc = scratchp.tile([P, L], f32)
                    # row totals via ScalarE accumulate (main out discarded)
                    nc.scalar.activation(sc[:], xt[:], AF.Copy, accum_out=ft[:, 0:1])
                    # edge columns + bias ones column on VectorE
                    nc.vector.tensor_copy(ft[:, 1:3], xt[:, 0:2])
                    nc.vector.tensor_copy(ft[:, 3:5], xt[:, L - 2 : L])
                    nc.vector.memset(ft[:, 5:6], 1.0)

                    ftT_p = ftp.tile([6, P], f32)
                    nc.tensor.transpose(ftT_p[:], ft[:], ident[:])
                    ftT = fts.tile([6, P], f32)
                    nc.vector.tensor_copy(ftT[:], ftT_p[:])

                    op = outp.tile([P, D], f32)
                    nc.tensor.matmul(op[:], ftT[:], m6[:])
                    ot = outs.tile([P, D], f32)
                    nc.vector.tensor_copy(ot[:], op[:])
                    nc.sync.dma_start(o_d[bass.ts(i, P), :], ot[:])

            if repeat == 1:
                body()
            else:
                with tc.For_i(0, repeat, 1):
                    body()

    nc.compile()
    return nc


def _host_m6(w: np.ndarray, b: np.ndarray) -> np.ndarray:
    w = w.astype(np.float32)
    invL = np.float32(1.0 / L)
    rows = [
        w.sum(axis=1) * invL,            # total
        -(w[:, 3] + w[:, 4]) * invL,     # x[0]
        -w[:, 4] * invL,                 # x[1]
        -w[:, 0] * invL,                 # x[L-2]
        -(w[:, 0] + w[:, 1]) * invL,     # x[L-1]
        b.astype(np.float32),            # ones
    ]
    return np.stack(rows).astype(np.float32)


def kernel(x: np.ndarray, w: np.ndarray, b: np.ndarray) -> np.ndarray:
    from concourse.bass_utils import run_bass_kernel_spmd

    if "nc" not in _CACHE:
        _CACHE["nc"] = _build()
    nc = _CACHE["nc"]

    m6 = _host_m6(w, b)
    ident = np.eye(P, dtype=np.float32)
    shards = np.ascontiguousarray(x.astype(np.float32).reshape(BT, L)).reshape(
        N_CORES, ROWS_PER_CORE, L
    )
    in_maps = [
        {"x": shards[i], "m6": m6, "ident": ident} for i in range(N_CORES)
    ]
    res = run_bass_kernel_spmd(nc, in_maps, list(range(N_CORES))).results
    out = np.concatenate([res[i]["out"] for i in range(N_CORES)], axis=0)
    return out.reshape(B, T, D)
